# revision 6
# baseline (speedup 1.0000x reference)
"""CLIP (InfoNCE) loss kernel for Trainium2, 8 NeuronCores.

loss = 0.5*(ce_m + ce_s) where
  ce_m = mean_i( LSE_j(l[i,:]) - l[i,i] ),  ce_s = mean_j( LSE_i(l[:,j]) - l[j,j] )
  l = logit_scale * (m @ s.T),  B=16384, D=256.

Strategy (data parallel on batch rows, 8 cores; core c owns rows
[c*2048, (c+1)*2048) of m and sees the full s):

  - PE: logits via fp8(e4m3) matmuls in DoubleRowSwInterleave perf mode —
    one K=256 pass per [128 x 512] psum panel at ~2x bf16 throughput. The
    weight operand is pre-interleaved on the host (pairs adjacent, columns
    reversed), which is what the SW-interleave mode expects and what keeps
    LDWEIGHTS off the critical path. Inputs are pre-scaled by
    sqrt(A' * |logit_scale|) each, A' = 128*log2(e), so psum holds A'*l
    directly (the Schraudolph affine needs it; ACT's free scale/bias
    undoes it for the exact-exp path).
  - exp(l - SHIFT) is split across two engines per 1536-column group:
      ACT: exact exp on cols [0:1024) with fused accum_out row-partials.
      DVE: Schraudolph bit-trick exp on cols [1024:1536):
           bits_u16 = (psum max C) + B2; bits reinterpreted as bf16 IS
           exp(l-SHIFT) up to a mean-zero sawtooth (sigma calibrated), with
           clamp C making underflow exact-zero-harmless and never negative.
      The two engines read disjoint psum tiles (psA/psD) and write separate
      SBUF tiles so nothing serializes.
  - row sums: ACT side fused (accum_out); DVE side via bf16 2x-mode
    elementwise accumulation across groups into per-rowblock acc tiles
    (one tensor_tensor add per tile), final reduction on the host.
  - column sums: ones-vector matmuls into 4 concurrent 32-column PE strips
    of one psum bank, accumulated over the 16 row-blocks of each group;
    the bank is opened/closed by rank-1 zero matmuls so all strips share
    one clean accumulation group. Drained via one DVE copy + DMA per group.
  - diag l[i,i] and all tiny final reductions/log/merges happen on host in
    float64 (O(B*D) and O(B) work).

SHIFT = 6*|scale|*sqrt(D): logits ~ N(0, (scale*sqrt(D))^2), so exp never
overflows and anything that underflows is ~e^-80 below the max — far below
f32 relative precision.
"""

import math
from contextlib import ExitStack

import numpy as np
import ml_dtypes

import concourse.bacc as bacc
import concourse.bass as bass
import concourse.tile as tile
from concourse import mybir
from concourse.bass_utils import run_bass_kernel_spmd

BF16 = ml_dtypes.bfloat16
FP8 = ml_dtypes.float8_e4m3

B = 16384
D = 256
NCORES = 8
ROWS = B // NCORES          # 2048 rows per core
P = 128
MT = ROWS // P              # 16 row-blocks per core
PN = 512                    # psum bank width (f32)

LOG2E = 1.4426950408889634
APRIME = 128.0 * LOG2E      # matmul pre-scale so psum = A' * logits
SIGMA = 0.05730129086530929  # Schraudolph sawtooth mean-zero offset (round-to-nearest)
CLAMP_BELOW = 85.0          # clamp logits below SHIFT-85 (contributes ~e^-85)

XF = 1024                   # ACT columns per full group (DVE gets WF-XF)
WF = 1536                   # full column-group width (3 psum banks)
NGF = 10                    # full groups
WL = B - NGF * WF           # last group width (1024)
NG = NGF + 1
DW = WF - XF                # DVE slice width (512, constant across groups)
XL = WL - DW                # ACT columns in the last group (512)

f32 = mybir.dt.float32
bf16 = mybir.dt.bfloat16
fp8 = mybir.dt.float8e4
u16 = mybir.dt.uint16

_nc_cache: dict[float, "bass.Bass"] = {}


def _build(shift: float, reps: int | None = None) -> "bass.Bass":
    nc = bacc.Bacc(trn_type="TRN2")

    mSwi_d = nc.dram_tensor("mSwi", [P, 2 * ROWS], fp8, kind="ExternalInput")
    sTa_d = nc.dram_tensor("sTa", [P, 2, B], fp8, kind="ExternalInput")

    rowsa_d = nc.dram_tensor("rowsa", [P, MT * NG], f32, kind="ExternalOutput")
    acc_d = nc.dram_tensor("acc", [MT * P, DW], bf16, kind="ExternalOutput")
    colsum_d = nc.dram_tensor("colsum", [NG * 4, WF // 4], f32, kind="ExternalOutput")

    # DVE Schraudolph constants: psum y = A'*l ; bits = (y max C) + B2
    C = APRIME * (shift - CLAMP_BELOW)
    B2 = 128.0 * (127.0 - SIGMA) - APRIME * shift

    with ExitStack() as ctx:
        tc = ctx.enter_context(tile.TileContext(nc))
        singles = ctx.enter_context(tc.tile_pool(name="singles", bufs=1))
        epool = ctx.enter_context(tc.tile_pool(name="epool", bufs=6))
        mainps = ctx.enter_context(tc.tile_pool(name="mainps", bufs=2, space="PSUM"))
        colps = ctx.enter_context(tc.tile_pool(name="colps", bufs=2, space="PSUM"))

        mSwi_sb = singles.tile([P, 2 * ROWS], fp8, tag="mSwi")
        nc.sync.dma_start(out=mSwi_sb, in_=mSwi_d[:, :])
        NCH = 8
        CW = B // NCH
        sTa_sb = singles.tile([P, 2, B], fp8, tag="sTa")
        for q in range(NCH):
            nc.sync.dma_start(
                out=sTa_sb[:, :, q * CW : (q + 1) * CW],
                in_=sTa_d[:, :, q * CW : (q + 1) * CW],
            )

        ones = singles.tile([P, 1], bf16, tag="ones")
        nc.vector.memset(ones, 1.0)
        negshift = singles.tile([P, 1], f32, tag="negshift")
        nc.vector.memset(negshift, -shift)
        z97 = singles.tile([1, 97], bf16, tag="z97")
        nc.vector.memset(z97, 0.0)
        zW4 = singles.tile([1, WF // 4], bf16, tag="zW4")
        nc.vector.memset(zW4, 0.0)

        rowsa_sb = singles.tile([P, MT * NG], f32, tag="rowsa")
        accs = [
            singles.tile([P, DW], bf16, name=f"acc{mt}", tag=f"acc{mt}")
            for mt in range(MT)
        ]

        def body():
            _emit_pass(nc, tc, epool, mainps, colps, mSwi_sb, sTa_sb, ones,
                       negshift, z97, zW4, rowsa_sb, accs, rowsa_d, acc_d,
                       colsum_d, C, B2)

        if reps is not None:
            with tc.For_i(0, reps):
                body()
        else:
            body()

    nc.compile()
    return nc


def _emit_pass(nc, tc, epool, mainps, colps, mSwi_sb, sTa_sb, ones, negshift,
               z97, zW4, rowsa_sb, accs, rowsa_d, acc_d, colsum_d, C, B2):
    if True:
        for g in range(NG):
            W = WF if g < NGF else WL
            x = XF if g < NGF else XL
            c0 = g * WF
            wa = x // 4
            wd = DW // 4
            colpsum = colps.tile([97, WF // 4], f32)
            # open one accumulation group covering the whole bank
            nc.tensor.matmul(
                colpsum, lhsT=z97, rhs=zW4, start=True, stop=False,
                skip_group_check=True,
            )

            def emit_strips(mt, ea, ed, colpsum=colpsum, wa=wa, wd=wd):
                for strip in range(4):
                    nc.tensor.matmul(
                        colpsum[32 * strip : 32 * strip + 1, 0:wa],
                        lhsT=ones,
                        rhs=ea[:, strip * wa : (strip + 1) * wa],
                        start=False, stop=False,
                        tile_position=(0, 32 * strip),
                        skip_group_check=True,
                    )
                for strip in range(4):
                    nc.tensor.matmul(
                        colpsum[32 * strip : 32 * strip + 1, (WF // 4) - wd :],
                        lhsT=ones,
                        rhs=ed[:, strip * wd : (strip + 1) * wd],
                        start=False, stop=False,
                        tile_position=(0, 32 * strip),
                        skip_group_check=True,
                    )

            pend = []
            for mt in range(MT):
                psA = mainps.tile([P, XF], f32, name="psA", tag="psA")
                psD = mainps.tile([P, WF - XF], f32, name="psD", tag="psD")
                for k in range(W // PN):
                    cc = k * PN
                    out = (
                        psA[:, cc : cc + PN]
                        if cc < x
                        else psD[:, cc - x : cc - x + PN]
                    )
                    nc.tensor.matmul(
                        out,
                        lhsT=mSwi_sb[:, mt * 256 : (mt + 1) * 256],
                        rhs=sTa_sb[:, :, c0 + k * PN : c0 + (k + 1) * PN],
                        start=True, stop=True,
                        perf_mode=mybir.MatmulPerfMode.DoubleRowSwInterleave,
                    )
                slot = mt * NG + g
                ea = epool.tile([P, XF], bf16, name="ea", tag="ea")
                ed = epool.tile([P, DW], bf16, name="ed", tag="ed")
                nc.scalar.activation(
                    ea[:, 0:x],
                    psA[:, 0:x],
                    mybir.ActivationFunctionType.Exp,
                    bias=negshift[:, 0:1],
                    scale=1.0 / APRIME,
                    accum_out=rowsa_sb[:, slot : slot + 1],
                )
                nc.vector.tensor_scalar(
                    ed.bitcast(u16),
                    psD[:, 0:DW],
                    C,
                    B2,
                    op0=mybir.AluOpType.max,
                    op1=mybir.AluOpType.add,
                )
                if g == 0:
                    nc.vector.tensor_copy(out=accs[mt], in_=ed)
                else:
                    nc.vector.tensor_tensor(
                        out=accs[mt], in0=accs[mt], in1=ed,
                        op=mybir.AluOpType.add,
                    )
                pend.append((mt, ea, ed))
                if len(pend) > 1:
                    emit_strips(*pend.pop(0))
            while pend:
                emit_strips(*pend.pop(0))
            nc.tensor.matmul(
                colpsum, lhsT=z97, rhs=zW4, start=False, stop=True,
                skip_group_check=True,
            )
            colsb = epool.tile([97, WF // 4], f32, tag="colsb")
            nc.vector.tensor_copy(out=colsb, in_=colpsum)
            for strip in range(4):
                nc.sync.dma_start(
                    out=colsum_d[4 * g + strip : 4 * g + strip + 1, :],
                    in_=colsb[32 * strip : 32 * strip + 1, :],
                )

        nc.sync.dma_start(out=rowsa_d[:, :], in_=rowsa_sb)
        for mt in range(MT):
            nc.sync.dma_start(out=acc_d[mt * P : (mt + 1) * P, :], in_=accs[mt])


def _get_nc(shift: float) -> "bass.Bass":
    if shift not in _nc_cache:
        _nc_cache[shift] = _build(shift)
    return _nc_cache[shift]


def make_in_maps(m, s, scale):
    """Host prep: fp8 pre-scaled operands; lhs in SW-interleave layout."""
    alpha = math.sqrt(APRIME * abs(scale)) if scale != 0.0 else 0.0
    sgn = 1.0 if scale >= 0 else -1.0
    mT = np.ascontiguousarray(
        (m.astype(np.float64) * (alpha * sgn)).T.astype(np.float32)
    )
    sT = np.ascontiguousarray((s.astype(np.float64) * alpha).T.astype(np.float32))
    # [D, n] -> [128, 2, n]: D-half index in the middle (DoubleRow layout)
    mTa = mT.reshape(2, P, B).transpose(1, 0, 2).astype(FP8)
    sTa = np.ascontiguousarray(sT.reshape(2, P, B).transpose(1, 0, 2)).astype(FP8)
    # SwInterleave weights: per 128-col block, stored[p, 2c+i] = logical[p, i, 127-c]
    swi = np.zeros((P, B * 2), FP8)
    cidx = np.arange(128)
    for blk in range(B // P):
        blkv = mTa[:, :, blk * P : (blk + 1) * P]
        swi[:, blk * 256 + 2 * cidx] = blkv[:, 0, 127 - cidx]
        swi[:, blk * 256 + 2 * cidx + 1] = blkv[:, 1, 127 - cidx]
    in_maps = []
    for c in range(NCORES):
        in_maps.append(
            {
                "mSwi": np.ascontiguousarray(
                    swi[:, c * ROWS * 2 : (c + 1) * ROWS * 2]
                ),
                "sTa": sTa,
            }
        )
    return in_maps


def host_finish(results, m, s, scale, shift):
    rowsum = np.zeros((NCORES, MT, P), np.float64)
    colsum = np.zeros(B, np.float64)
    W4f = WF // 4
    for c, r in enumerate(results):
        ra = r["rowsa"].astype(np.float64)          # [P, MT*NG]
        rowsum[c] += ra.reshape(P, MT, NG).sum(axis=2).T
        rowsum[c] += r["acc"].astype(np.float64).reshape(MT, P, -1).sum(axis=2)
        cs = r["colsum"].astype(np.float64)         # [NG*4, WF//4]
        for g in range(NG):
            W = WF if g < NGF else WL
            x = XF if g < NGF else XL
            wa, wd = x // 4, (W - x) // 4
            for strip in range(4):
                ja = g * WF + strip * wa
                colsum[ja : ja + wa] += cs[4 * g + strip, 0:wa]
                jd = g * WF + x + strip * wd
                colsum[jd : jd + wd] += cs[4 * g + strip, W4f - wd : W4f]
    rowsum = rowsum.reshape(B)
    diag = (m.astype(np.float64) * s.astype(np.float64)).sum(axis=1) * float(scale)
    rowlse = shift + np.log(rowsum)
    collse = shift + np.log(colsum)
    loss = np.mean(0.5 * (rowlse + collse) - diag)
    return np.float32(loss)


def run(inputs: dict, trace: bool = False):
    m = np.asarray(inputs["modality_features"], dtype=np.float32)
    s = np.asarray(inputs["sequence_features"], dtype=np.float32)
    scale = float(np.asarray(inputs["logit_scale"], dtype=np.float32))
    assert m.shape == (B, D) and s.shape == (B, D)

    shift = float(6.0 * abs(scale) * math.sqrt(D))
    nc = _get_nc(shift)
    in_maps = make_in_maps(m, s, scale)
    res = run_bass_kernel_spmd(nc, in_maps, list(range(NCORES)), trace=trace)
    loss = host_finish(res.results, m, s, scale, shift)
    return np.asarray(loss, dtype=np.float32), res


def kernel(**inputs) -> np.ndarray:
    out, _ = run(inputs, trace=False)
    return out



# revision 7
# speedup vs baseline: 1.0805x; 1.0805x over previous
"""CLIP (InfoNCE) loss kernel for Trainium2, 8 NeuronCores.

loss = 0.5*(ce_m + ce_s) where
  ce_m = mean_i( LSE_j(l[i,:]) - l[i,i] ),  ce_s = mean_j( LSE_i(l[:,j]) - l[j,j] )
  l = logit_scale * (m @ s.T),  B=16384, D=256.

Strategy (data parallel on batch rows, 8 cores; core c owns rows
[c*2048, (c+1)*2048) of m and sees the full s):

  - PE: logits via fp8(e4m3) matmuls in DoubleRowSwInterleave perf mode —
    one K=256 pass per [128 x 512] psum panel at ~2x bf16 throughput. The
    weight operand is pre-interleaved on the host (pairs adjacent, columns
    reversed), which is what the SW-interleave mode expects and what keeps
    LDWEIGHTS off the critical path. Inputs are pre-scaled by
    sqrt(A' * |logit_scale|) each, A' = 128*log2(e), so psum holds A'*l
    directly (the Schraudolph affine needs it; ACT's free scale/bias
    undoes it for the exact-exp path).
  - exp(l - SHIFT) is split across two engines per 1536-column group:
      ACT: exact exp on cols [0:1024) with fused accum_out row-partials.
      DVE: Schraudolph bit-trick exp on cols [1024:1536):
           bits_u16 = (psum max C) + B2; bits reinterpreted as bf16 IS
           exp(l-SHIFT) up to a mean-zero sawtooth (sigma calibrated), with
           clamp C making underflow exact-zero-harmless and never negative.
      The two engines read disjoint psum tiles (psA/psD) and write separate
      SBUF tiles so nothing serializes.
  - row sums: ACT side fused (accum_out); DVE side via bf16 2x-mode
    elementwise accumulation across groups into per-rowblock acc tiles
    (one tensor_tensor add per tile; two rowblocks' chains run on the
    otherwise-idle Pool engine), final reduction on the host. The acc
    tiles are zero-initialized by DMA from a zeros input at the start of
    each pass, which keeps the first-touch copy off the DVE.
  - column sums: ones-vector matmuls into 4 concurrent 32-column PE strips
    of one psum bank, accumulated over the 16 row-blocks of each group;
    the bank is opened/closed by rank-1 zero matmuls so all strips share
    one clean accumulation group. Drained via one DVE copy + DMA per group.
  - diag l[i,i] and all tiny final reductions/log/merges happen on host in
    float64 (O(B*D) and O(B) work).

SHIFT = 6*|scale|*sqrt(D): logits ~ N(0, (scale*sqrt(D))^2), so exp never
overflows and anything that underflows is ~e^-80 below the max — far below
f32 relative precision.
"""

import math
from contextlib import ExitStack

import numpy as np
import ml_dtypes

import concourse.bacc as bacc
import concourse.bass as bass
import concourse.tile as tile
from concourse import mybir
from concourse.bass_utils import run_bass_kernel_spmd

BF16 = ml_dtypes.bfloat16
FP8 = ml_dtypes.float8_e4m3

B = 16384
D = 256
NCORES = 8
ROWS = B // NCORES          # 2048 rows per core
P = 128
MT = ROWS // P              # 16 row-blocks per core
PN = 512                    # psum bank width (f32)

LOG2E = 1.4426950408889634
APRIME = 128.0 * LOG2E      # matmul pre-scale so psum = A' * logits
SIGMA = 0.05730129086530929  # Schraudolph sawtooth mean-zero offset (round-to-nearest)
CLAMP_BELOW = 85.0          # clamp logits below SHIFT-85 (contributes ~e^-85)

XF = 1024                   # ACT columns per full group (DVE gets WF-XF)
WF = 1536                   # full column-group width (3 psum banks)
NGF = 10                    # full groups
WL = B - NGF * WF           # last group width (1024)
NG = NGF + 1
DW = WF - XF                # DVE slice width (512, constant across groups)
XL = WL - DW                # ACT columns in the last group (512)

f32 = mybir.dt.float32
bf16 = mybir.dt.bfloat16
fp8 = mybir.dt.float8e4
u16 = mybir.dt.uint16

_nc_cache: dict[float, "bass.Bass"] = {}


def _build(shift: float, reps: int | None = None) -> "bass.Bass":
    nc = bacc.Bacc(trn_type="TRN2")

    mSwi_d = nc.dram_tensor("mSwi", [P, 2 * ROWS], fp8, kind="ExternalInput")
    sTa_d = nc.dram_tensor("sTa", [P, 2, B], fp8, kind="ExternalInput")
    zacc_d = nc.dram_tensor("zacc", [P, DW], bf16, kind="ExternalInput")

    rowsa_d = nc.dram_tensor("rowsa", [P, MT * NG], f32, kind="ExternalOutput")
    acc_d = nc.dram_tensor("acc", [MT * P, DW], bf16, kind="ExternalOutput")
    colsum_d = nc.dram_tensor("colsum", [NG * 4, WF // 4], f32, kind="ExternalOutput")

    # DVE Schraudolph constants: psum y = A'*l ; bits = (y max C) + B2
    C = APRIME * (shift - CLAMP_BELOW)
    B2 = 128.0 * (127.0 - SIGMA) - APRIME * shift

    with ExitStack() as ctx:
        tc = ctx.enter_context(tile.TileContext(nc))
        singles = ctx.enter_context(tc.tile_pool(name="singles", bufs=1))
        epool = ctx.enter_context(tc.tile_pool(name="epool", bufs=6))
        mainps = ctx.enter_context(tc.tile_pool(name="mainps", bufs=2, space="PSUM"))
        colps = ctx.enter_context(tc.tile_pool(name="colps", bufs=2, space="PSUM"))

        mSwi_sb = singles.tile([P, 2 * ROWS], fp8, tag="mSwi")
        nc.sync.dma_start(out=mSwi_sb, in_=mSwi_d[:, :])
        NCH = 8
        CW = B // NCH
        sTa_sb = singles.tile([P, 2, B], fp8, tag="sTa")
        for q in range(NCH):
            nc.sync.dma_start(
                out=sTa_sb[:, :, q * CW : (q + 1) * CW],
                in_=sTa_d[:, :, q * CW : (q + 1) * CW],
            )

        ones = singles.tile([P, 1], bf16, tag="ones")
        nc.vector.memset(ones, 1.0)
        negshift = singles.tile([P, 1], f32, tag="negshift")
        nc.vector.memset(negshift, -shift)
        z97 = singles.tile([1, 97], bf16, tag="z97")
        nc.vector.memset(z97, 0.0)
        zW4 = singles.tile([1, WF // 4], bf16, tag="zW4")
        nc.vector.memset(zW4, 0.0)

        rowsa_sb = singles.tile([P, MT * NG], f32, tag="rowsa")
        accs = [
            singles.tile([P, DW], bf16, name=f"acc{mt}", tag=f"acc{mt}")
            for mt in range(MT)
        ]

        def body():
            _emit_pass(nc, tc, epool, mainps, colps, mSwi_sb, sTa_sb, ones,
                       negshift, z97, zW4, rowsa_sb, accs, rowsa_d, acc_d,
                       colsum_d, C, B2, zacc_d)

        if reps is not None:
            with tc.For_i(0, reps):
                body()
        else:
            body()

    nc.compile()
    return nc


POOL_MTS = frozenset({0, 8})   # accs chains on Pool


def _emit_pass(nc, tc, epool, mainps, colps, mSwi_sb, sTa_sb, ones, negshift,
               z97, zW4, rowsa_sb, accs, rowsa_d, acc_d, colsum_d, C, B2,
               zacc_d):
    if True:
        for mt in range(MT):
            nc.sync.dma_start(out=accs[mt], in_=zacc_d[:, :])
        for g in range(NG):
            W = WF if g < NGF else WL
            x = XF if g < NGF else XL
            c0 = g * WF
            wa = x // 4
            wd = DW // 4
            colpsum = colps.tile([97, WF // 4], f32)
            # open one accumulation group covering the whole bank
            nc.tensor.matmul(
                colpsum, lhsT=z97, rhs=zW4, start=True, stop=False,
                skip_group_check=True,
            )

            def emit_strips(mt, ea, ed, colpsum=colpsum, wa=wa, wd=wd):
                for strip in range(4):
                    nc.tensor.matmul(
                        colpsum[32 * strip : 32 * strip + 1, 0:wa],
                        lhsT=ones,
                        rhs=ea[:, strip * wa : (strip + 1) * wa],
                        start=False, stop=False,
                        tile_position=(0, 32 * strip),
                        skip_group_check=True,
                    )
                for strip in range(4):
                    nc.tensor.matmul(
                        colpsum[32 * strip : 32 * strip + 1, (WF // 4) - wd :],
                        lhsT=ones,
                        rhs=ed[:, strip * wd : (strip + 1) * wd],
                        start=False, stop=False,
                        tile_position=(0, 32 * strip),
                        skip_group_check=True,
                    )

            pend = []
            for mt in range(MT):
                psA = mainps.tile([P, XF], f32, name="psA", tag="psA")
                psD = mainps.tile([P, WF - XF], f32, name="psD", tag="psD")
                for k in range(W // PN):
                    cc = k * PN
                    out = (
                        psA[:, cc : cc + PN]
                        if cc < x
                        else psD[:, cc - x : cc - x + PN]
                    )
                    nc.tensor.matmul(
                        out,
                        lhsT=mSwi_sb[:, mt * 256 : (mt + 1) * 256],
                        rhs=sTa_sb[:, :, c0 + k * PN : c0 + (k + 1) * PN],
                        start=True, stop=True,
                        perf_mode=mybir.MatmulPerfMode.DoubleRowSwInterleave,
                    )
                slot = mt * NG + g
                ea = epool.tile([P, XF], bf16, name="ea", tag="ea")
                ed = epool.tile([P, DW], bf16, name="ed", tag="ed")
                nc.scalar.activation(
                    ea[:, 0:x],
                    psA[:, 0:x],
                    mybir.ActivationFunctionType.Exp,
                    bias=negshift[:, 0:1],
                    scale=1.0 / APRIME,
                    accum_out=rowsa_sb[:, slot : slot + 1],
                )
                nc.vector.tensor_scalar(
                    ed.bitcast(u16),
                    psD[:, 0:DW],
                    C,
                    B2,
                    op0=mybir.AluOpType.max,
                    op1=mybir.AluOpType.add,
                )
                eng = nc.gpsimd if mt in POOL_MTS else nc.vector
                eng.tensor_tensor(
                    out=accs[mt], in0=accs[mt], in1=ed,
                    op=mybir.AluOpType.add,
                )
                pend.append((mt, ea, ed))
                if len(pend) > 1:
                    emit_strips(*pend.pop(0))
            while pend:
                emit_strips(*pend.pop(0))
            nc.tensor.matmul(
                colpsum, lhsT=z97, rhs=zW4, start=False, stop=True,
                skip_group_check=True,
            )
            colsb = epool.tile([97, WF // 4], f32, tag="colsb")
            nc.vector.tensor_copy(out=colsb, in_=colpsum)
            for strip in range(4):
                nc.sync.dma_start(
                    out=colsum_d[4 * g + strip : 4 * g + strip + 1, :],
                    in_=colsb[32 * strip : 32 * strip + 1, :],
                )

        nc.sync.dma_start(out=rowsa_d[:, :], in_=rowsa_sb)
        for mt in range(MT):
            nc.sync.dma_start(out=acc_d[mt * P : (mt + 1) * P, :], in_=accs[mt])


def _get_nc(shift: float) -> "bass.Bass":
    if shift not in _nc_cache:
        _nc_cache[shift] = _build(shift)
    return _nc_cache[shift]


def make_in_maps(m, s, scale):
    """Host prep: fp8 pre-scaled operands; lhs in SW-interleave layout."""
    alpha = math.sqrt(APRIME * abs(scale)) if scale != 0.0 else 0.0
    sgn = 1.0 if scale >= 0 else -1.0
    mT = np.ascontiguousarray(
        (m.astype(np.float64) * (alpha * sgn)).T.astype(np.float32)
    )
    sT = np.ascontiguousarray((s.astype(np.float64) * alpha).T.astype(np.float32))
    # [D, n] -> [128, 2, n]: D-half index in the middle (DoubleRow layout)
    mTa = mT.reshape(2, P, B).transpose(1, 0, 2).astype(FP8)
    sTa = np.ascontiguousarray(sT.reshape(2, P, B).transpose(1, 0, 2)).astype(FP8)
    # SwInterleave weights: per 128-col block, stored[p, 2c+i] = logical[p, i, 127-c]
    swi = np.zeros((P, B * 2), FP8)
    cidx = np.arange(128)
    for blk in range(B // P):
        blkv = mTa[:, :, blk * P : (blk + 1) * P]
        swi[:, blk * 256 + 2 * cidx] = blkv[:, 0, 127 - cidx]
        swi[:, blk * 256 + 2 * cidx + 1] = blkv[:, 1, 127 - cidx]
    in_maps = []
    for c in range(NCORES):
        in_maps.append(
            {
                "mSwi": np.ascontiguousarray(
                    swi[:, c * ROWS * 2 : (c + 1) * ROWS * 2]
                ),
                "sTa": sTa,
                "zacc": np.zeros((P, DW), BF16),
            }
        )
    return in_maps


def host_finish(results, m, s, scale, shift):
    rowsum = np.zeros((NCORES, MT, P), np.float64)
    colsum = np.zeros(B, np.float64)
    W4f = WF // 4
    for c, r in enumerate(results):
        ra = r["rowsa"].astype(np.float64)          # [P, MT*NG]
        rowsum[c] += ra.reshape(P, MT, NG).sum(axis=2).T
        rowsum[c] += r["acc"].astype(np.float64).reshape(MT, P, -1).sum(axis=2)
        cs = r["colsum"].astype(np.float64)         # [NG*4, WF//4]
        for g in range(NG):
            W = WF if g < NGF else WL
            x = XF if g < NGF else XL
            wa, wd = x // 4, (W - x) // 4
            for strip in range(4):
                ja = g * WF + strip * wa
                colsum[ja : ja + wa] += cs[4 * g + strip, 0:wa]
                jd = g * WF + x + strip * wd
                colsum[jd : jd + wd] += cs[4 * g + strip, W4f - wd : W4f]
    rowsum = rowsum.reshape(B)
    diag = (m.astype(np.float64) * s.astype(np.float64)).sum(axis=1) * float(scale)
    rowlse = shift + np.log(rowsum)
    collse = shift + np.log(colsum)
    loss = np.mean(0.5 * (rowlse + collse) - diag)
    return np.float32(loss)


def run(inputs: dict, trace: bool = False):
    m = np.asarray(inputs["modality_features"], dtype=np.float32)
    s = np.asarray(inputs["sequence_features"], dtype=np.float32)
    scale = float(np.asarray(inputs["logit_scale"], dtype=np.float32))
    assert m.shape == (B, D) and s.shape == (B, D)

    shift = float(6.0 * abs(scale) * math.sqrt(D))
    nc = _get_nc(shift)
    in_maps = make_in_maps(m, s, scale)
    res = run_bass_kernel_spmd(nc, in_maps, list(range(NCORES)), trace=trace)
    loss = host_finish(res.results, m, s, scale, shift)
    return np.asarray(loss, dtype=np.float32), res


def kernel(**inputs) -> np.ndarray:
    out, _ = run(inputs, trace=False)
    return out



# revision 8
# speedup vs baseline: 1.0829x; 1.0022x over previous
"""CLIP (InfoNCE) loss kernel for Trainium2, 8 NeuronCores.

loss = 0.5*(ce_m + ce_s) where
  ce_m = mean_i( LSE_j(l[i,:]) - l[i,i] ),  ce_s = mean_j( LSE_i(l[:,j]) - l[j,j] )
  l = logit_scale * (m @ s.T),  B=16384, D=256.

Strategy (data parallel on batch rows, 8 cores; core c owns rows
[c*2048, (c+1)*2048) of m and sees the full s):

  - PE: logits via fp8(e4m3) matmuls in DoubleRowSwInterleave perf mode —
    one K=256 pass per [128 x 512] psum panel at ~2x bf16 throughput. The
    weight operand is pre-interleaved on the host (pairs adjacent, columns
    reversed), which is what the SW-interleave mode expects and what keeps
    LDWEIGHTS off the critical path. Inputs are pre-scaled by
    sqrt(A' * |logit_scale|) each, A' = 128*log2(e), so psum holds A'*l
    directly (the Schraudolph affine needs it; ACT's free scale/bias
    undoes it for the exact-exp path).
  - exp(l - SHIFT) is split across two engines per 1536-column group:
      ACT: exact exp on cols [0:1024) with fused accum_out row-partials.
      DVE: Schraudolph bit-trick exp on cols [1024:1536):
           bits_u16 = (psum max C) + B2; bits reinterpreted as bf16 IS
           exp(l-SHIFT) up to a mean-zero sawtooth (sigma calibrated), with
           clamp C making underflow exact-zero-harmless and never negative.
      The two engines read disjoint psum tiles (psA/psD) and write separate
      SBUF tiles so nothing serializes.
  - row sums: ACT side fused (accum_out); DVE side via bf16 2x-mode
    elementwise accumulation across groups into per-rowblock acc tiles
    (one tensor_tensor add per tile; two rowblocks' chains run on the
    otherwise-idle Pool engine), final reduction on the host. The acc
    tiles are zero-initialized by DMA from a zeros input at the start of
    each pass, which keeps the first-touch copy off the DVE.
  - column sums: ones-vector matmuls into 4 concurrent 32-column PE strips
    of one psum bank, accumulated over the 16 row-blocks of each group;
    the bank is opened/closed by rank-1 zero matmuls so all strips share
    one clean accumulation group. Drained via one DVE copy + DMA per group.
  - diag l[i,i] and all tiny final reductions/log/merges happen on host in
    float64 (O(B*D) and O(B) work).

SHIFT = 6*|scale|*sqrt(D): logits ~ N(0, (scale*sqrt(D))^2), so exp never
overflows and anything that underflows is ~e^-80 below the max — far below
f32 relative precision.
"""

import math
from contextlib import ExitStack

import numpy as np
import ml_dtypes

import concourse.bacc as bacc
import concourse.bass as bass
import concourse.tile as tile
from concourse import mybir
from concourse.bass_utils import run_bass_kernel_spmd

BF16 = ml_dtypes.bfloat16
FP8 = ml_dtypes.float8_e4m3

B = 16384
D = 256
NCORES = 8
ROWS = B // NCORES          # 2048 rows per core
P = 128
MT = ROWS // P              # 16 row-blocks per core
PN = 512                    # psum bank width (f32)

LOG2E = 1.4426950408889634
APRIME = 128.0 * LOG2E      # matmul pre-scale so psum = A' * logits
SIGMA = 0.05730129086530929  # Schraudolph sawtooth mean-zero offset (round-to-nearest)
CLAMP_BELOW = 85.0          # clamp logits below SHIFT-85 (contributes ~e^-85)

XF = 1024                   # ACT columns per full group (DVE gets WF-XF)
WF = 1536                   # full column-group width (3 psum banks)
NGF = 10                    # full groups
WL = B - NGF * WF           # last group width (1024)
NG = NGF + 1
DW = WF - XF                # DVE slice width (512, constant across groups)
XL = WL - DW                # ACT columns in the last group (512)

f32 = mybir.dt.float32
bf16 = mybir.dt.bfloat16
fp8 = mybir.dt.float8e4
u16 = mybir.dt.uint16

_nc_cache: dict[float, "bass.Bass"] = {}


def _build(shift: float, reps: int | None = None) -> "bass.Bass":
    nc = bacc.Bacc(trn_type="TRN2")

    mSwi_d = nc.dram_tensor("mSwi", [P, 2 * ROWS], fp8, kind="ExternalInput")
    sTa_d = nc.dram_tensor("sTa", [P, 2, B], fp8, kind="ExternalInput")
    zacc_d = nc.dram_tensor("zacc", [P, DW], bf16, kind="ExternalInput")

    rowsa_d = nc.dram_tensor("rowsa", [P, MT * NG], f32, kind="ExternalOutput")
    acc_d = nc.dram_tensor("acc", [MT * P, DW], bf16, kind="ExternalOutput")
    colsum_d = nc.dram_tensor("colsum", [NG * 4, WF // 4], f32, kind="ExternalOutput")

    # DVE Schraudolph constants: psum y = A'*l ; bits = (y max C) + B2
    C = APRIME * (shift - CLAMP_BELOW)
    B2 = 128.0 * (127.0 - SIGMA) - APRIME * shift

    with ExitStack() as ctx:
        tc = ctx.enter_context(tile.TileContext(nc))
        singles = ctx.enter_context(tc.tile_pool(name="singles", bufs=1))
        epool = ctx.enter_context(tc.tile_pool(name="epool", bufs=6))
        mainps = ctx.enter_context(tc.tile_pool(name="mainps", bufs=2, space="PSUM"))
        colps = ctx.enter_context(tc.tile_pool(name="colps", bufs=2, space="PSUM"))

        mSwi_sb = singles.tile([P, 2 * ROWS], fp8, tag="mSwi")
        nc.sync.dma_start(out=mSwi_sb, in_=mSwi_d[:, :])
        NCH = 8
        CW = B // NCH
        sTa_sb = singles.tile([P, 2, B], fp8, tag="sTa")
        for q in range(NCH):
            nc.sync.dma_start(
                out=sTa_sb[:, :, q * CW : (q + 1) * CW],
                in_=sTa_d[:, :, q * CW : (q + 1) * CW],
            )

        ones = singles.tile([P, 1], bf16, tag="ones")
        nc.vector.memset(ones, 1.0)
        negshift = singles.tile([P, 1], f32, tag="negshift")
        nc.vector.memset(negshift, -shift)
        z97 = singles.tile([1, 97], bf16, tag="z97")
        nc.vector.memset(z97, 0.0)
        zW4 = singles.tile([1, WF // 4], bf16, tag="zW4")
        nc.vector.memset(zW4, 0.0)

        rowsa_sb = singles.tile([P, MT * NG], f32, tag="rowsa")
        accs = [
            singles.tile([P, DW], bf16, name=f"acc{mt}", tag=f"acc{mt}")
            for mt in range(MT)
        ]

        def body():
            _emit_pass(nc, tc, epool, mainps, colps, mSwi_sb, sTa_sb, ones,
                       negshift, z97, zW4, rowsa_sb, accs, rowsa_d, acc_d,
                       colsum_d, C, B2, zacc_d)

        if reps is not None:
            with tc.For_i(0, reps):
                body()
        else:
            body()

    nc.compile()
    return nc


POOL_MTS = frozenset({4, 12})   # accs chains on Pool


def _emit_pass(nc, tc, epool, mainps, colps, mSwi_sb, sTa_sb, ones, negshift,
               z97, zW4, rowsa_sb, accs, rowsa_d, acc_d, colsum_d, C, B2,
               zacc_d):
    if True:
        for mt in range(MT):
            nc.sync.dma_start(out=accs[mt], in_=zacc_d[:, :])
        for g in range(NG):
            W = WF if g < NGF else WL
            x = XF if g < NGF else XL
            c0 = g * WF
            wa = x // 4
            wd = DW // 4
            colpsum = colps.tile([97, WF // 4], f32)
            # open one accumulation group covering the whole bank
            nc.tensor.matmul(
                colpsum, lhsT=z97, rhs=zW4, start=True, stop=False,
                skip_group_check=True,
            )

            def emit_strips(mt, ea, ed, colpsum=colpsum, wa=wa, wd=wd):
                for strip in range(4):
                    nc.tensor.matmul(
                        colpsum[32 * strip : 32 * strip + 1, 0:wa],
                        lhsT=ones,
                        rhs=ea[:, strip * wa : (strip + 1) * wa],
                        start=False, stop=False,
                        tile_position=(0, 32 * strip),
                        skip_group_check=True,
                    )
                for strip in range(4):
                    nc.tensor.matmul(
                        colpsum[32 * strip : 32 * strip + 1, (WF // 4) - wd :],
                        lhsT=ones,
                        rhs=ed[:, strip * wd : (strip + 1) * wd],
                        start=False, stop=False,
                        tile_position=(0, 32 * strip),
                        skip_group_check=True,
                    )

            pend = []
            for mt in range(MT):
                psA = mainps.tile([P, XF], f32, name="psA", tag="psA")
                psD = mainps.tile([P, WF - XF], f32, name="psD", tag="psD")
                for k in range(W // PN):
                    cc = k * PN
                    out = (
                        psA[:, cc : cc + PN]
                        if cc < x
                        else psD[:, cc - x : cc - x + PN]
                    )
                    nc.tensor.matmul(
                        out,
                        lhsT=mSwi_sb[:, mt * 256 : (mt + 1) * 256],
                        rhs=sTa_sb[:, :, c0 + k * PN : c0 + (k + 1) * PN],
                        start=True, stop=True,
                        perf_mode=mybir.MatmulPerfMode.DoubleRowSwInterleave,
                    )
                slot = mt * NG + g
                ea = epool.tile([P, XF], bf16, name="ea", tag="ea")
                ed = epool.tile([P, DW], bf16, name="ed", tag="ed")
                nc.scalar.activation(
                    ea[:, 0:x],
                    psA[:, 0:x],
                    mybir.ActivationFunctionType.Exp,
                    bias=negshift[:, 0:1],
                    scale=1.0 / APRIME,
                    accum_out=rowsa_sb[:, slot : slot + 1],
                )
                nc.vector.tensor_scalar(
                    ed.bitcast(u16),
                    psD[:, 0:DW],
                    C,
                    B2,
                    op0=mybir.AluOpType.max,
                    op1=mybir.AluOpType.add,
                )
                eng = nc.gpsimd if mt in POOL_MTS else nc.vector
                eng.tensor_tensor(
                    out=accs[mt], in0=accs[mt], in1=ed,
                    op=mybir.AluOpType.add,
                )
                pend.append((mt, ea, ed))
                if len(pend) > 1:
                    emit_strips(*pend.pop(0))
            while pend:
                emit_strips(*pend.pop(0))
            nc.tensor.matmul(
                colpsum, lhsT=z97, rhs=zW4, start=False, stop=True,
                skip_group_check=True,
            )
            colsb = epool.tile([97, WF // 4], f32, tag="colsb")
            nc.vector.tensor_copy(out=colsb, in_=colpsum)
            for strip in range(4):
                nc.sync.dma_start(
                    out=colsum_d[4 * g + strip : 4 * g + strip + 1, :],
                    in_=colsb[32 * strip : 32 * strip + 1, :],
                )

        nc.sync.dma_start(out=rowsa_d[:, :], in_=rowsa_sb)
        for mt in range(MT):
            nc.sync.dma_start(out=acc_d[mt * P : (mt + 1) * P, :], in_=accs[mt])


def _get_nc(shift: float) -> "bass.Bass":
    if shift not in _nc_cache:
        _nc_cache[shift] = _build(shift)
    return _nc_cache[shift]


def make_in_maps(m, s, scale):
    """Host prep: fp8 pre-scaled operands; lhs in SW-interleave layout."""
    alpha = math.sqrt(APRIME * abs(scale)) if scale != 0.0 else 0.0
    sgn = 1.0 if scale >= 0 else -1.0
    mT = np.ascontiguousarray(
        (m.astype(np.float64) * (alpha * sgn)).T.astype(np.float32)
    )
    sT = np.ascontiguousarray((s.astype(np.float64) * alpha).T.astype(np.float32))
    # [D, n] -> [128, 2, n]: D-half index in the middle (DoubleRow layout)
    mTa = mT.reshape(2, P, B).transpose(1, 0, 2).astype(FP8)
    sTa = np.ascontiguousarray(sT.reshape(2, P, B).transpose(1, 0, 2)).astype(FP8)
    # SwInterleave weights: per 128-col block, stored[p, 2c+i] = logical[p, i, 127-c]
    swi = np.zeros((P, B * 2), FP8)
    cidx = np.arange(128)
    for blk in range(B // P):
        blkv = mTa[:, :, blk * P : (blk + 1) * P]
        swi[:, blk * 256 + 2 * cidx] = blkv[:, 0, 127 - cidx]
        swi[:, blk * 256 + 2 * cidx + 1] = blkv[:, 1, 127 - cidx]
    in_maps = []
    for c in range(NCORES):
        in_maps.append(
            {
                "mSwi": np.ascontiguousarray(
                    swi[:, c * ROWS * 2 : (c + 1) * ROWS * 2]
                ),
                "sTa": sTa,
                "zacc": np.zeros((P, DW), BF16),
            }
        )
    return in_maps


def host_finish(results, m, s, scale, shift):
    rowsum = np.zeros((NCORES, MT, P), np.float64)
    colsum = np.zeros(B, np.float64)
    W4f = WF // 4
    for c, r in enumerate(results):
        ra = r["rowsa"].astype(np.float64)          # [P, MT*NG]
        rowsum[c] += ra.reshape(P, MT, NG).sum(axis=2).T
        rowsum[c] += r["acc"].astype(np.float64).reshape(MT, P, -1).sum(axis=2)
        cs = r["colsum"].astype(np.float64)         # [NG*4, WF//4]
        for g in range(NG):
            W = WF if g < NGF else WL
            x = XF if g < NGF else XL
            wa, wd = x // 4, (W - x) // 4
            for strip in range(4):
                ja = g * WF + strip * wa
                colsum[ja : ja + wa] += cs[4 * g + strip, 0:wa]
                jd = g * WF + x + strip * wd
                colsum[jd : jd + wd] += cs[4 * g + strip, W4f - wd : W4f]
    rowsum = rowsum.reshape(B)
    diag = (m.astype(np.float64) * s.astype(np.float64)).sum(axis=1) * float(scale)
    rowlse = shift + np.log(rowsum)
    collse = shift + np.log(colsum)
    loss = np.mean(0.5 * (rowlse + collse) - diag)
    return np.float32(loss)


def run(inputs: dict, trace: bool = False):
    m = np.asarray(inputs["modality_features"], dtype=np.float32)
    s = np.asarray(inputs["sequence_features"], dtype=np.float32)
    scale = float(np.asarray(inputs["logit_scale"], dtype=np.float32))
    assert m.shape == (B, D) and s.shape == (B, D)

    shift = float(6.0 * abs(scale) * math.sqrt(D))
    nc = _get_nc(shift)
    in_maps = make_in_maps(m, s, scale)
    res = run_bass_kernel_spmd(nc, in_maps, list(range(NCORES)), trace=trace)
    loss = host_finish(res.results, m, s, scale, shift)
    return np.asarray(loss, dtype=np.float32), res


def kernel(**inputs) -> np.ndarray:
    out, _ = run(inputs, trace=False)
    return out

